# revision 79
# baseline (speedup 1.0000x reference)
"""3-layer GCN (GCNConv + residual + relu, global add pool, MLP softmax) on 8
Trainium2 NeuronCores.

Design: nodes sharded by range; EDGES PARTITIONED BY SOURCE core, so every
message's source row is core-local and no feature AllGather is needed.
Per layer, each core:
  phase A: xw = (dinv*h) @ Wg written node-major into a LOCAL DRAM table,
    plus a feature-major copy (the self-loop term) kept in SBUF.
  phase B: dma_gather per-edge source rows from the local table (int16
    indices), segment-sum them into per-64-dst-window PSUM accumulators via
    one-hot matmuls. The fp16 one-hots are built on the DVE at its 2x rate
    (dstrel slabs vs a host "staircase" constant; window-crossing extras
    use host-prepared shifted dstrel columns). Node positions within each
    dst shard are PERMUTED by a greedy balancer so the per-(source core,
    window) message counts are nearly equal across cores, and tiles may
    cross superblock boundaries (PSUM double-buffering keeps both alive),
    so the slot stream (padded to the max over cores so all 8 cores run
    the IDENTICAL program) carries ~2% padding instead of ~12%.
  Superblock PSUM is drained (Act) to fp8 staging and written into a
    full-size [8*128 feat-rows x cols] partial-aggregate table; one
    ReduceScatter per column-split (fp8, add) sums partials across cores
    and hands each core its own shard feature-major — exactly the epilogue
    layout. RS is charged by OUTPUT size in the cost model, so it is ~7x
    cheaper than AllGathering message tables. Collectives are only
    hardware-valid on the gpsimd queue (which also carries the gathers and
    serializes per-queue in the cost model), so both RS's run back-to-back
    AFTER the uninterrupted gather stream; the first split's epilogue +
    next-layer phase A hide under the second RS on the other queues, and
    only the last, smaller (65/35) split's epilogue is a true tail, with
    its stages spread across the otherwise-idle queues (t1/u2 on gpsimd,
    drains on DVE/Act).
  epilogue: h = relu(h + dinv*(agg + xwfm) + bg), chunked.
Pooled [64,128] partials ride one more ReduceScatter over an 8x-replicated
input (each core's shard = the full cross-core sum, feature-major, feeding
the classifier matmul directly); the tiny classifier is replicated. All cores run the IDENTICAL program; per-core
variation lives entirely in data (gather indices, sel values, padding).
"""
import numpy as np
import ml_dtypes

import concourse.bacc as bacc
import concourse.bass as bass
import concourse.mybir as mybir
import concourse.tile as tile
from concourse.bass_utils import run_bass_kernel_spmd

NCORES = 8
G = 64     # graphs in batch (pooled rows)
C = 2      # classes
WINW = 64   # dst window width (one-hot sel columns per matmul)
SBCOLS = 896  # columns per superblock (psum granularity)
NSPLIT = 2  # column splits per layer, one ReduceScatter each
SLAB = 16  # sel tiles built per is_equal op

bf16 = ml_dtypes.bfloat16
_cache = {}


def _ceil(a, b):
    return -(-a // b)


# --------------------------------------------------------------------------
# host preprocessing
# --------------------------------------------------------------------------
def _preprocess(x, edge_index, batch):
    N, D = x.shape
    assert D == 128 and N % NCORES == 0
    NLOC = N // NCORES
    NPAD = _ceil(NLOC, 128) * 128
    NW = NPAD // WINW
    sbw = min(SBCOLS // WINW, NW)
    if NW // sbw < 2:
        sbw = max(1, NW // 2)
    while NW % sbw:
        sbw -= 1
    nsb_total = NW // sbw
    # uneven splits: early RS's finish while gathers still stream (their
    # epilogues hide); the last split is small so the exposed tail is short
    if nsb_total >= NSPLIT:
        fr = [0.65, 0.35][:NSPLIT]
        sb_per = []
        left = nsb_total
        for i, f in enumerate(fr):
            n = left - (NSPLIT - 1 - i) if i == len(fr) - 1 else \
                max(1, min(left - (NSPLIT - 1 - i), int(round(nsb_total * f))))
            sb_per.append(n)
            left -= n
        sb_per[-1] += left
    else:
        sb_per = [1] * nsb_total
    WIN = [n * sbw for n in sb_per]                  # windows per split
    WOFF = [0]
    for n in WIN[:-1]:
        WOFF.append(WOFF[-1] + n)
    NSPL = len(WIN)
    NWF = NCORES * NW           # dst windows across all shards
    NSBF = NWF // sbw

    src = np.asarray(edge_index[0], np.int64)
    dst = np.asarray(edge_index[1], np.int64)
    deg = np.bincount(dst, minlength=N).astype(np.float64) + 1.0
    dinv = (deg ** -0.5).astype(np.float32)

    src_f = src
    dst_f = dst

    mc = src_f // NLOC                  # owning core (message source)

    # --- balance per-(source core, dst window) counts by permuting node
    # positions within each dst shard (the slot cap is the max over cores,
    # so equalizing the 8 per-core loads kills most of the padding) ---
    cd0 = dst_f // NLOC
    cnt8 = np.zeros((N, NCORES), np.int32)
    np.add.at(cnt8, (dst_f, mc), 1)
    newpos = np.empty(N, np.int64)
    capw = NPAD // WINW
    for cc in range(NCORES):
        nodes = np.arange(cc * NLOC, (cc + 1) * NLOC)
        v = cnt8[nodes]                         # [NLOC, 8]
        order_n = np.argsort(-v.sum(1), kind="stable")
        load = np.zeros((capw, NCORES), np.int64)
        room = np.full(capw, WINW, np.int64)
        room[-1] -= NPAD - NLOC                 # padded tail slots
        wsel = np.empty(NLOC, np.int64)
        for idx in order_n:
            cand = np.max(load + v[idx], axis=1).astype(np.float64)
            cand[room <= 0] = np.inf
            wbest = int(np.argmin(cand))
            wsel[idx] = wbest
            load[wbest] += v[idx]
            room[wbest] -= 1
        # positions: fill each window with its nodes
        off = np.zeros(capw, np.int64)
        woff = np.r_[0, np.cumsum(np.full(capw, WINW))][:-1]
        fill = np.zeros(capw, np.int64)
        pos = np.empty(NLOC, np.int64)
        for idx in np.argsort(wsel, kind="stable"):
            w_ = wsel[idx]
            pos[idx] = woff[w_] + fill[w_]
            fill[w_] += 1
        newpos[nodes] = pos
    # node -> padded local position (0..NPAD); invert for data layouts
    sl = newpos[src_f]                  # gather index into local table
    cd = cd0
    dl = newpos[dst_f]
    wl = dl // WINW                     # local dst window 0..NW-1
    bounds = np.array(WOFF + [NW])
    sp = np.searchsorted(bounds, wl, side="right") - 1  # column split
    winarr = np.array(WIN)
    woffarr = np.array(WOFF)
    # stream window order: split -> dst core -> local window
    swi = (NCORES * woffarr[sp] + cd * winarr[sp] + (wl - woffarr[sp]))
    drel = dl - wl * WINW

    cnt = np.bincount(mc * NWF + swi, minlength=NCORES * NWF
                      ).reshape(NCORES, NWF)
    cap = np.maximum(cnt.max(axis=0), 1)  # identical stream on every core

    # group chunking pattern (sbs per gather group), per split; the last
    # group is a single superblock so the RS's drain-wait tail is short
    chunk_pat = {}
    for spc in range(NSPL):
        nsb_blk = WIN[spc] // sbw
        if nsb_blk <= 1:
            chunk_pat[spc] = [(0, nsb_blk)]
            continue
        body = nsb_blk - 1
        ch = [(i, min(i + 4, body)) for i in range(0, body, 4)]
        ch.append((body, nsb_blk))
        chunk_pat[spc] = ch

    # slot stream: windows in swi order; tiles may cross superblock
    # boundaries (pad to x128 only at gather-group boundaries)
    win_off = np.zeros(NWF, np.int64)
    slot_w_list = []
    group_spans = []        # (sp, cd, t0, ntiles, [sb indices])
    so = 0
    for spc in range(NSPL):
        nsb_blk = WIN[spc] // sbw
        for cdc in range(NCORES):
            b0 = (NCORES * WOFF[spc] + cdc * WIN[spc]) // sbw
            for lo, hi in chunk_pat[spc]:
                t0 = so // 128
                for sb in range(b0 + lo, b0 + hi):
                    for j in range(sbw):
                        w = sb * sbw + j
                        win_off[w] = so
                        slot_w_list.append(np.full(int(cap[w]), w, np.int64))
                        so += int(cap[w])
                pad = (-so) % 128
                if pad:
                    slot_w_list.append(np.full(pad, -1, np.int64))
                    so += pad
                group_spans.append((spc, cdc, t0, so // 128 - t0,
                                    list(range(b0 + lo, b0 + hi))))
    SLOTS = so
    NT = SLOTS // 128
    slot_w = np.concatenate(slot_w_list)

    # tiles -> MM list (tile, stream window, iota_k); start/stop per window
    first_w = np.empty(NT, np.int64)
    mm_list = []
    for t in range(NT):
        ws_here = slot_w[t * 128:(t + 1) * 128]
        ws_u = np.unique(ws_here[ws_here >= 0])
        fw = int(ws_u[0]) if len(ws_u) else int(slot_w[t * 128 - 1])
        first_w[t] = fw
        for w_ in ws_u:
            k = int(w_ - fw)
            assert 0 <= k < 4 * sbw
            mm_list.append((t, int(w_), k))
    firstmm = np.full(NWF, -1, np.int64)
    lastmm = np.full(NWF, -1, np.int64)
    for i, (t, w_, k) in enumerate(mm_list):
        if firstmm[w_] < 0:
            firstmm[w_] = i
        lastmm[w_] = i
    assert (firstmm >= 0).all(), "every window needs at least one MM"
    ex_cols = []
    mm_flags = []
    for i, (t, w_, k) in enumerate(mm_list):
        e = -1
        if k > 0:
            e = len(ex_cols)
            ex_cols.append((t, k))
        mm_flags.append((t, w_, k, i == firstmm[w_], i == lastmm[w_], e))
    NEX = len(ex_cols)
    NEXP = _ceil(max(NEX, 1), SLAB) * SLAB

    # gather groups from the stream spans; per-sb drain info
    groups = []   # dicts: sp, cd, t0, nt, sbs [(sb, wl0, lasttile)]
    for (spc, cdc, t0, nt, sbl) in group_spans:
        info = []
        for sb in sbl:
            wlo = sb * sbw
            whi = wlo + sbw
            lastt = -1
            for w in range(wlo, whi):
                lt = int((win_off[w] + cap[w] - 1) // 128)
                lastt = max(lastt, lt)
            b0w = (NCORES * WOFF[spc] + cdc * WIN[spc]) // sbw
            wl0 = WOFF[spc] + (sb - b0w) * sbw
            info.append((sb, wl0, cdc, lastt))
        groups.append(dict(sp=spc, cd=cdc, t0=t0, nt=nt, sbs=info))

    # per-core slot placement
    order = np.lexsort((swi, mc))
    mc_s = mc[order]
    keyall = mc_s * NWF + swi[order]
    starts = np.r_[0, np.flatnonzero(np.diff(keyall)) + 1]
    gid = np.zeros(len(keyall), np.int64)
    gid[starts[1:]] = 1
    gid = np.cumsum(gid)
    pos = np.arange(len(keyall)) - starts[gid]
    slot = win_off[swi[order]] + pos
    assert (pos < cap[swi[order]]).all()

    gidx_all = np.zeros((NCORES, SLOTS), np.int16)
    dstrel_all = np.full((NCORES, SLOTS), -1.0, np.float32)
    gidx_all[mc_s, slot] = sl[order].astype(np.int16)
    dstrel_all[mc_s, slot] = ((swi[order] - first_w[slot // 128]) * WINW
                              + drel[order]).astype(np.float32)
    assert (dstrel_all[mc_s, slot] >= 0).all()
    assert dstrel_all.max() < sbw * WINW

    gidx_dev = np.tile(
        gidx_all.reshape(NCORES, SLOTS // 16, 16).transpose(0, 2, 1), (1, 8, 1)
    ).copy()                                        # [8, 128, SLOTS//16]
    dstrel_dev = dstrel_all.reshape(NCORES, NT, 128).transpose(0, 2, 1).copy()

    dstrel_ex_dev = np.full((NCORES, 128, NEXP), -1000.0, np.float32)
    for e, (t, k) in enumerate(ex_cols):
        dstrel_ex_dev[:, :, e] = dstrel_dev[:, :, t] - float(WINW) * k

    batch = np.asarray(batch, np.int64)
    brel = np.full((NCORES, NPAD), -1.0, np.float32)
    x = np.asarray(x, np.float32)
    xt_dev = np.zeros((NCORES, 128, NPAD), bf16)
    dinvT_dev = np.zeros((NCORES, 128, NPAD), bf16)
    for cc in range(NCORES):
        nodes = np.arange(cc * NLOC, (cc + 1) * NLOC)
        posl = newpos[nodes]
        brel[cc, posl] = batch[nodes]
        xt_dev[cc][:, posl] = x[nodes].T.astype(bf16)
        dv = np.zeros(NPAD, np.float32)
        dv[posl] = dinv[nodes]
        dinvT_dev[cc] = np.broadcast_to(dv.astype(bf16), (128, NPAD))
    batchrel_dev = brel.reshape(NCORES, NPAD // 128, 128
                                ).transpose(0, 2, 1).copy()

    meta = dict(N=N, NLOC=NLOC, NPAD=NPAD, NW=NW, WIN=WIN, WOFF=WOFF,
                NWF=NWF, NT=NT, SLOTS=SLOTS, groups=groups,
                mm_flags=mm_flags, NEX=NEX, NEXP=NEXP, SBWE=sbw,
                NSPL=len(WIN))
    data = dict(gidx=gidx_dev, dstrel=dstrel_dev, batchrel=batchrel_dev,
                xt=xt_dev, dinvt=dinvT_dev, dstrel_ex=dstrel_ex_dev)
    return meta, data


# --------------------------------------------------------------------------
# device program
# --------------------------------------------------------------------------
def _build(meta, L):
    f32 = mybir.dt.float32
    b16 = mybir.dt.bfloat16
    f16 = mybir.dt.float16
    fp8 = mybir.dt.float8e4
    i16 = mybir.dt.int16
    NPAD, NW = meta["NPAD"], meta["NW"]
    WIN, WOFF = meta["WIN"], meta["WOFF"]
    NSPL = meta["NSPL"]
    NT, SLOTS = meta["NT"], meta["SLOTS"]
    SBWE = meta["SBWE"]
    NEXP = meta["NEXP"]
    groups, mm_flags = meta["groups"], meta["mm_flags"]
    rg = [list(range(NCORES))]
    mm_by_tile = {}
    for (t, w_, k, st_f, sp_f, e) in mm_flags:
        mm_by_tile.setdefault(t, []).append((w_, k, st_f, sp_f, e))

    nc = bacc.Bacc("TRN2", target_bir_lowering=False, debug=False,
                   num_devices=NCORES)
    d_xt = nc.dram_tensor("xt", [128, NPAD], b16, kind="ExternalInput")
    d_dinvt = nc.dram_tensor("dinvt", [128, NPAD], b16, kind="ExternalInput")
    d_gidx = nc.dram_tensor("gidx", [128, SLOTS // 16], i16, kind="ExternalInput")
    d_dstrel = nc.dram_tensor("dstrel", [128, NT], f16, kind="ExternalInput")
    NWP = NPAD // 128
    d_batchrel = nc.dram_tensor("batchrel", [128, NWP], f16,
                                kind="ExternalInput")
    d_w0 = nc.dram_tensor("w0", [128, 128], b16, kind="ExternalInput")
    d_wg = nc.dram_tensor("wg", [L, 128, 128], b16, kind="ExternalInput")
    d_wc1 = nc.dram_tensor("wc1", [128, 128], b16, kind="ExternalInput")
    d_wc2 = nc.dram_tensor("wc2", [128, C], b16, kind="ExternalInput")
    d_b0 = nc.dram_tensor("b0", [128, 1], f32, kind="ExternalInput")
    d_bg = nc.dram_tensor("bg", [L, 128, 1], f32, kind="ExternalInput")
    d_bc1 = nc.dram_tensor("bc1", [128, 1], f32, kind="ExternalInput")
    d_bc2m = nc.dram_tensor("bc2m", [G, C], f32, kind="ExternalInput")
    d_stair = nc.dram_tensor("stair", [128, 128 * SLAB], f16,
                             kind="ExternalInput")
    d_dstrel_ex = nc.dram_tensor("dstrel_ex", [128, NEXP], f16,
                                 kind="ExternalInput")
    d_id128 = nc.dram_tensor("id128", [128, 128], b16, kind="ExternalInput")
    d_idg = nc.dram_tensor("idg", [G, G], b16, kind="ExternalInput")
    d_out = nc.dram_tensor("out", [G, C], f32, kind="ExternalOutput")

    xw_loc = [nc.dram_tensor(f"xw_loc{l}", [NPAD, 128], b16)
              for l in range(L)]
    CSPL = [WIN[s] * WINW for s in range(NSPL)]
    rs_in = [[nc.dram_tensor(f"rs_in{l}_{s}", [NCORES * 128, CSPL[s]], fp8)
              for s in range(NSPL)] for l in range(L)]
    rs_out = [[nc.dram_tensor(f"rs_out{l}_{s}", [128, CSPL[s]], fp8)
               for s in range(NSPL)] for l in range(L)]
    pool_in = nc.dram_tensor("pool_in", [NCORES * G, 128], b16)
    pool_out = nc.dram_tensor("pool_out", [G, 128], b16)

    Relu = mybir.ActivationFunctionType.Relu
    Exp = mybir.ActivationFunctionType.Exp
    Copy = mybir.ActivationFunctionType.Copy
    AT = mybir.AluOpType

    with tile.TileContext(nc) as tc:
        with (
            tc.tile_pool(name="state", bufs=1) as state,
            tc.tile_pool(name="wpool", bufs=1) as wpool,
            tc.tile_pool(name="xin", bufs=3) as xinp,
            tc.tile_pool(name="xws", bufs=3) as xwsp,
            tc.tile_pool(name="xwn", bufs=3) as xwnp,
            tc.tile_pool(name="gix", bufs=6) as gixp,
            tc.tile_pool(name="gbf", bufs=3) as gbfp,
            tc.tile_pool(name="sel", bufs=8) as selp,
            tc.tile_pool(name="stg", bufs=8) as stgp,
            tc.tile_pool(name="epi", bufs=6) as epip,
            tc.tile_pool(name="psxw", bufs=2, space="PSUM") as psxw,
            tc.tile_pool(name="pstr", bufs=2, space="PSUM") as pstr,
            tc.tile_pool(name="pswin", bufs=2, space="PSUM") as pswin,
        ):
            # ---- persistent state + constants ----
            h = state.tile([128, NPAD], b16, tag="h")
            dinvT = state.tile([128, NPAD], b16, tag="dinvT")
            xwfm = state.tile([128, NPAD], fp8, tag="xwfm")
            dstrel = state.tile([128, NT], f16, tag="dstrel")

            w0 = wpool.tile([128, 128], b16, tag="w0")
            nc.sync.dma_start(w0[:], d_w0[:])
            b0 = wpool.tile([128, 1], f32, tag="b0")
            nc.sync.dma_start(b0[:], d_b0[:])
            wg = wpool.tile([128, L, 128], b16, tag="wg")
            nc.sync.dma_start(wg[:], d_wg.rearrange("l p f -> p l f"))
            nc.gpsimd.dma_start(dinvT[:], d_dinvt[:])
            nc.gpsimd.dma_start(dstrel[:], d_dstrel[:])
            wc1 = wpool.tile([128, 128], b16, tag="wc1")
            nc.scalar.dma_start(wc1[:], d_wc1[:])
            wc2 = wpool.tile([128, C], b16, tag="wc2")
            nc.scalar.dma_start(wc2[:], d_wc2[:])
            bg = wpool.tile([128, L], f32, tag="bg")
            nc.scalar.dma_start(bg[:], d_bg.rearrange("l p o -> p (l o)"))
            bc1 = wpool.tile([128, 1], f32, tag="bc1")
            nc.scalar.dma_start(bc1[:], d_bc1[:])
            bc2m = wpool.tile([G, C], f32, tag="bc2m")
            nc.scalar.dma_start(bc2m[:], d_bc2m[:])
            stair = wpool.tile([128, 128 * SLAB], f16, tag="stair")
            nc.gpsimd.dma_start(stair[:], d_stair[:])
            dstrel_ex = wpool.tile([128, NEXP], f16, tag="dstrel_ex")
            nc.gpsimd.dma_start(dstrel_ex[:], d_dstrel_ex[:])
            id128 = wpool.tile([128, 128], b16, tag="id128")
            nc.sync.dma_start(id128[:], d_id128[:])
            idg = wpool.tile([G, G], b16, tag="idg")
            nc.scalar.dma_start(idg[:], d_idg[:])
            batchrel = wpool.tile([128, NWP], f16, tag="batchrel")
            nc.gpsimd.dma_start(batchrel[:], d_batchrel[:])

            def emit_phaseA_cols(l, c0, c1hi, dve_drain=False):
                """xw_loc[l] rows [c0,c1hi) (node-major) from current h,
                plus the feature-major copy (self-loop term) into xwfm.
                Table writes batched per pair of 512-col chunks."""
                while c0 < c1hi:
                    cb = min(512, c1hi - c0)
                    nw_ = cb // 128
                    xwn = xwnp.tile([128, nw_, 128], b16, tag="xwn",
                                    name="xwn")
                    cc0 = c0
                    while cc0 < c0 + cb:
                        cw = min(512, c0 + cb - cc0)
                        hs = xwsp.tile([128, cw], b16, tag="xws", name="hs")
                        nc.vector.tensor_tensor(out=hs[:],
                                                in0=h[:, cc0:cc0 + cw],
                                                in1=dinvT[:, cc0:cc0 + cw],
                                                op=AT.mult)
                        ps = psxw.tile([128, cw], f32, tag="psxw", name="ps")
                        for j in range(cw // 128):
                            nc.tensor.matmul(ps[:, j * 128:(j + 1) * 128],
                                             lhsT=hs[:, j * 128:(j + 1) * 128],
                                             rhs=wg[:, l, :],
                                             start=True, stop=True)
                        jo = (cc0 - c0) // 128
                        psv = bass.AP(ps.tensor, ps[:].offset,
                                      [ps[:].ap[0], [128, cw // 128],
                                       [1, 128]])
                        dstv = bass.AP(xwn.tensor,
                                       xwn[:, jo:jo + cw // 128, :].offset,
                                       [xwn[:].ap[0], [128, cw // 128],
                                        [1, 128]])
                        if dve_drain:
                            nc.vector.tensor_copy(out=dstv, in_=psv)
                        else:
                            nc.scalar.activation(out=dstv, in_=psv, func=Copy)
                        ps2 = psxw.tile([128, cw], f32, tag="psxw", name="ps2")
                        nc.tensor.matmul(ps2[:], lhsT=wg[:, l, :],
                                         rhs=hs[:], start=True, stop=True)
                        nc.scalar.activation(out=xwfm[:, cc0:cc0 + cw],
                                             in_=ps2[:], func=Copy)
                        cc0 += cw
                    nc.sync.dma_start(
                        bass.AP(xw_loc[l], c0 * 128,
                                [[128, 128], [128 * 128, nw_], [1, 128]]),
                        xwn[:])
                    c0 += cb

            # ---- stage 1: h = relu(W0.T @ xT + b0) ----
            nchunks = _ceil(NPAD, 512)
            for kk in range(nchunks):
                c0 = kk * 512
                cw = min(512, NPAD - c0)
                xts = xinp.tile([128, cw], b16, tag="xts", name="xts")
                nc.sync.dma_start(xts[:], d_xt[:, c0:c0 + cw])
                ps = psxw.tile([128, cw], f32, tag="psxw", name="ps")
                nc.tensor.matmul(ps[:], lhsT=w0[:], rhs=xts[:],
                                 start=True, stop=True)
                nc.scalar.activation(out=h[:, c0:c0 + cw], in_=ps[:],
                                     func=Relu, bias=b0[:])
            emit_phaseA_cols(0, 0, NPAD, dve_drain=True)

            # ---- GCN layers ----
            gsrc = [bass.AP(xw_loc[l], 0, [[128, NPAD], [1, 128]])
                    for l in range(L)]
            for l in range(L):
                sel_tiles = {}
                ex_tiles = {}
                ps_sb = {}

                def get_sel(t):
                    s = t // SLAB
                    if s not in sel_tiles:
                        t0 = s * SLAB
                        tn = min(SLAB, NT - t0)
                        st = selp.tile([128, WINW, tn], f16, tag="sel",
                                       name="st")
                        in0 = bass.AP(dstrel.tensor,
                                      dstrel[:, t0:t0 + tn].offset,
                                      [dstrel[:].ap[0], [0, WINW], [1, tn]])
                        in1 = bass.AP(stair.tensor, stair[:].offset,
                                      [stair[:].ap[0], [SLAB, WINW], [1, tn]])
                        nc.vector.tensor_tensor(out=st[:], in0=in0, in1=in1,
                                                op=AT.is_equal)
                        sel_tiles.clear()
                        sel_tiles[s] = (st, tn)
                    st, tn = sel_tiles[s]
                    return st, t - s * SLAB, tn

                def get_ex(e):
                    s = e // SLAB
                    if s not in ex_tiles:
                        e0 = s * SLAB
                        en = min(SLAB, NEXP - e0)
                        sx = selp.tile([128, WINW, en], f16, tag="selx",
                                       name="sx", bufs=4)
                        in0 = bass.AP(dstrel_ex.tensor,
                                      dstrel_ex[:, e0:e0 + en].offset,
                                      [dstrel_ex[:].ap[0], [0, WINW], [1, en]])
                        in1 = bass.AP(stair.tensor, stair[:].offset,
                                      [stair[:].ap[0], [SLAB, WINW], [1, en]])
                        nc.vector.tensor_tensor(out=sx[:], in0=in0, in1=in1,
                                                op=AT.is_equal)
                        ex_tiles.clear()
                        ex_tiles[s] = (sx, en)
                    sx, en = ex_tiles[s]
                    return sx, e - s * SLAB, en

                def emit_group(gr):
                    t0g, ntg = gr["t0"], gr["nt"]
                    slots = ntg * 128
                    so = t0g * 128
                    gixt = gixp.tile([128, slots // 16], i16, tag="gix")
                    nc.scalar.dma_start(
                        gixt[:], d_gidx[:, so // 16:(so + slots) // 16])
                    gb = gbfp.tile([128, ntg, 128], b16, tag="gbf")
                    nc.gpsimd.dma_gather(
                        gb[:], gsrc[l], gixt[:], slots, slots,
                        128, elem_step=128, single_packet=False)
                    nsb = len(gr["sbs"])
                    stg = stgp.tile([128, nsb, SBWE * WINW], fp8, tag="stg",
                                    name="stg")
                    drain_at = {}
                    sb_w0 = {}
                    for si_, (sbi, wl0, cdc, lastt) in enumerate(gr["sbs"]):
                        drain_at.setdefault(lastt, []).append(si_)
                        sb_w0[sbi] = sbi * SBWE
                    spg = gr["sp"]
                    for ti in range(t0g, t0g + ntg):
                        st, si, tn = get_sel(ti)
                        for (w_, k, st_f, sp_f, e) in mm_by_tile.get(ti, []):
                            sbi = w_ // SBWE
                            if sbi not in ps_sb:
                                ps_sb[sbi] = pswin.tile(
                                    [128, SBWE * WINW], f32,
                                    name="pswin_t", tag="pswin")
                            wr = w_ - sb_w0[sbi]
                            if k == 0:
                                rhs = bass.AP(
                                    st.tensor, st[:].offset + si,
                                    [st[:].ap[0], [tn, WINW]])
                            else:
                                sx, se, en = get_ex(e)
                                rhs = bass.AP(
                                    sx.tensor, sx[:].offset + se,
                                    [sx[:].ap[0], [en, WINW]])
                            nc.tensor.matmul(
                                ps_sb[sbi][:, wr * WINW:(wr + 1) * WINW],
                                lhsT=gb[:, ti - t0g, :], rhs=rhs,
                                start=bool(st_f), stop=bool(sp_f))
                        for si_ in drain_at.get(ti, []):
                            (sbi, wl0, cdc, lastt) = gr["sbs"][si_]
                            nc.scalar.activation(out=stg[:, si_, :],
                                                 in_=ps_sb.pop(sbi)[:],
                                                 func=Copy)
                    # one write for the whole group (contiguous rs_in cols)
                    wl00 = gr["sbs"][0][1]
                    cdc = gr["sbs"][0][2]
                    nc.sync.dma_start(
                        bass.AP(rs_in[l][spg],
                                cdc * 128 * CSPL[spg]
                                + (wl00 - WOFF[spg]) * WINW,
                                [[CSPL[spg], 128], [1, nsb * SBWE * WINW]]),
                        stg[:])

                # pool state for last layer
                pool_st = {}
                if l == L - 1:
                    pool_st["psp"] = psxw.tile([G, 128], f32, tag="psxw",
                                               name="psp")

                def emit_pool(wlo, whi):
                    # wlo/whi in 128-column units
                    for a in range(wlo, whi):
                        pstt = pstr.tile([128, 128], b16, tag="pstr",
                                         name="pst2")
                        nc.tensor.transpose(
                            pstt[:], h[:, a * 128:(a + 1) * 128], id128[:])
                        hn = epip.tile([128, 128], b16, tag="hn", name="hn",
                                       bufs=3)
                        nc.vector.tensor_copy(out=hn[:], in_=pstt[:])
                        if a % SLAB == 0:
                            a0 = a
                            an = min(SLAB, NWP - a0)
                            bsel = selp.tile([128, G, an], f16, tag="sel",
                                             name="bsel")
                            in0 = bass.AP(
                                batchrel.tensor,
                                batchrel[:, a0:a0 + an].offset,
                                [batchrel[:].ap[0], [0, G], [1, an]])
                            in1 = bass.AP(
                                stair.tensor, stair[:].offset,
                                [stair[:].ap[0], [SLAB, G], [1, an]])
                            nc.vector.tensor_tensor(
                                out=bsel[:], in0=in0, in1=in1,
                                op=AT.is_equal)
                            pool_st["bsel"] = (bsel, a0, an)
                        bsel, a0, an = pool_st["bsel"]
                        blhs = bass.AP(bsel.tensor,
                                       bsel[:].offset + (a - a0),
                                       [bsel[:].ap[0], [an, G]])
                        nc.tensor.matmul(pool_st["psp"][:], lhsT=blhs,
                                         rhs=hn[:], start=(a == 0),
                                         stop=(a == NWP - 1))

                def emit_epi_chunk(s, c0, cw, tail=False):
                    """One epilogue chunk: h cols [c0,c0+cw) = relu(h +
                    dinv*(agg+xwfm) + bg); then next-layer phase A (or
                    pooling) for those columns. In the exposed tail, spread
                    stages onto the otherwise-idle Pool queue."""
                    cs0 = WOFF[s] * WINW
                    if True:
                        agg = epip.tile([128, cw], fp8, tag="agg", name="agg",
                                        bufs=2)
                        nc.sync.dma_start(
                            agg[:], rs_out[l][s][:, c0 - cs0:c0 - cs0 + cw])
                        t1 = epip.tile([128, cw], b16, tag="t1", name="t1",
                                       bufs=2)
                        eng = nc.gpsimd if tail else nc.vector
                        eng.tensor_tensor(out=t1[:], in0=agg[:],
                                          in1=xwfm[:, c0:c0 + cw],
                                          op=AT.add)
                        emit_epi_chunk.u2eng = eng
                        u = epip.tile([128, cw], b16, tag="u", name="u",
                                      bufs=2)
                        nc.vector.tensor_tensor(out=u[:], in0=t1[:],
                                                in1=dinvT[:, c0:c0 + cw],
                                                op=AT.mult)
                        u2 = epip.tile([128, cw], b16, tag="u2", name="u2",
                                       bufs=2)
                        emit_epi_chunk.u2eng.tensor_tensor(
                            out=u2[:], in0=u[:], in1=h[:, c0:c0 + cw],
                            op=AT.add)
                        nc.scalar.activation(out=h[:, c0:c0 + cw], in_=u2[:],
                                             func=Relu, bias=bg[:, l:l + 1])
                        if l + 1 < L:
                            emit_phaseA_cols(l + 1, c0, c0 + cw,
                                             dve_drain=tail)
                        else:
                            emit_pool(c0 // 128, (c0 + cw) // 128)

                def epi_chunks(s, ch=None):
                    cs0 = WOFF[s] * WINW
                    CH = ch or SBWE * WINW
                    out = []
                    c0 = cs0
                    while c0 < cs0 + WIN[s] * WINW:
                        cw = min(CH, cs0 + WIN[s] * WINW - c0)
                        out.append((s, c0, cw))
                        c0 += cw
                    return out

                gs = [[gr for gr in groups if gr["sp"] == s]
                      for s in range(NSPL)]
                # all gathers first (uninterrupted Pool stream), then the
                # RS's back-to-back; epilogue s hides under RS s+1 on the
                # other queues (tail=False keeps its stages off Pool), and
                # only the last epilogue is a true tail
                for s in range(NSPL):
                    for gr in gs[s]:
                        emit_group(gr)
                for s in range(NSPL):
                    nc.gpsimd.collective_compute(
                        "ReduceScatter", AT.add,
                        ins=[rs_in[l][s][:]], outs=[rs_out[l][s][:]],
                        replica_groups=rg)
                    for ck in epi_chunks(s):
                        emit_epi_chunk(*ck, tail=(s == NSPL - 1))

                # ---- global add pool tail + classifier ----
                if l == L - 1:
                    pool_sb = epip.tile([G, NCORES, 128], b16,
                                        tag="poolsb", bufs=1)
                    nc.vector.tensor_copy(
                        out=pool_sb[:],
                        in_=bass.AP(pool_st["psp"].tensor,
                                    pool_st["psp"][:].offset,
                                    [pool_st["psp"][:].ap[0], [0, NCORES],
                                     [1, 128]]))
                    nc.sync.dma_start(
                        bass.AP(pool_in, 0,
                                [[128, G], [G * 128, NCORES], [1, 128]]),
                        pool_sb[:])
                    # RS over the 8x-replicated partials: every core's shard
                    # equals the full cross-core pooled sum
                    nc.gpsimd.collective_compute(
                        "ReduceScatter", AT.add, ins=[pool_in[:]],
                        outs=[pool_out[:]], replica_groups=rg)
                    pooledT = epip.tile([128, G], b16, tag="pooledT", bufs=1)
                    nc.sync.dma_start(
                        pooledT[:],
                        bass.AP(pool_out, 0, [[1, 128], [128, G]]))
                    psz = pstr.tile([128, G], f32, tag="pstr")
                    nc.tensor.matmul(psz[:], lhsT=wc1[:], rhs=pooledT[:],
                                     start=True, stop=True)
                    zt = epip.tile([128, G], b16, tag="zt", bufs=1)
                    nc.scalar.activation(out=zt[:], in_=psz[:], func=Relu,
                                         bias=bc1[:])
                    pslg = pstr.tile([G, C], f32, tag="pstr")
                    nc.tensor.matmul(pslg[:], lhsT=zt[:], rhs=wc2[:],
                                     start=True, stop=True)
                    lg = epip.tile([G, C], f32, tag="lg", bufs=1)
                    nc.vector.tensor_tensor(out=lg[:], in0=pslg[:],
                                            in1=bc2m[:], op=AT.add)
                    mx = epip.tile([G, 1], f32, tag="mx", bufs=1)
                    nc.vector.tensor_reduce(out=mx[:], in_=lg[:],
                                            axis=mybir.AxisListType.X,
                                            op=AT.max)
                    nmx = epip.tile([G, 1], f32, tag="nmx", bufs=1)
                    nc.vector.tensor_scalar_mul(nmx[:], mx[:], -1.0)
                    ex = epip.tile([G, C], f32, tag="ex", bufs=1)
                    nc.scalar.activation(out=ex[:], in_=lg[:], func=Exp,
                                         bias=nmx[:])
                    sm = epip.tile([G, 1], f32, tag="sm", bufs=1)
                    nc.vector.tensor_reduce(out=sm[:], in_=ex[:],
                                            axis=mybir.AxisListType.X,
                                            op=AT.add)
                    rs = epip.tile([G, 1], f32, tag="rs", bufs=1)
                    nc.vector.reciprocal(rs[:], sm[:])
                    prob = epip.tile([G, C], f32, tag="prob", bufs=1)
                    nc.vector.tensor_scalar_mul(prob[:], ex[:], rs[:])
                    nc.sync.dma_start(d_out[:], prob[:])

    nc.compile()
    return nc


# --------------------------------------------------------------------------
# entry point
# --------------------------------------------------------------------------
def kernel(x, edge_index, batch, W0, b0, Wg, bg, Wc1, bc1, Wc2, bc2,
           **extra):
    x = np.asarray(x, np.float32)
    edge_index = np.asarray(edge_index)
    batch = np.asarray(batch)
    W0 = np.asarray(W0, np.float32)
    Wg = np.asarray(Wg, np.float32)
    L = Wg.shape[0]

    key = (x.shape, edge_index.shape,
           hash(edge_index.tobytes()), hash(np.asarray(batch).tobytes()))
    if key not in _cache:
        meta, data = _preprocess(x, edge_index, batch)
        nc = _build(meta, L)
        _cache.clear()
        _cache[key] = (meta, data, nc)
    meta, data, nc = _cache[key]

    stair = np.broadcast_to(
        np.repeat(np.arange(128, dtype=np.float16), SLAB),
        (128, 128 * SLAB)).copy()
    common = dict(
        w0=W0.astype(bf16).view(np.uint16),
        wg=Wg.astype(bf16).view(np.uint16),
        wc1=np.asarray(Wc1, np.float32).astype(bf16).view(np.uint16),
        wc2=np.asarray(Wc2, np.float32).astype(bf16).view(np.uint16),
        b0=np.asarray(b0, np.float32).reshape(128, 1),
        bg=np.asarray(bg, np.float32).reshape(L, 128, 1),
        bc1=np.asarray(bc1, np.float32).reshape(128, 1),
        bc2m=np.broadcast_to(np.asarray(bc2, np.float32), (G, C)).copy(),
        stair=stair,
        id128=np.eye(128, dtype=np.float32).astype(bf16).view(np.uint16),
        idg=np.eye(G, dtype=np.float32).astype(bf16).view(np.uint16),
    )
    in_maps = []
    for c in range(NCORES):
        m = dict(common)
        m["xt"] = data["xt"][c].view(np.uint16)
        m["dinvt"] = data["dinvt"][c].view(np.uint16)
        m["gidx"] = data["gidx"][c]
        m["dstrel"] = data["dstrel"][c].astype(np.float16)
        m["dstrel_ex"] = data["dstrel_ex"][c].astype(np.float16)
        m["batchrel"] = data["batchrel"][c].astype(np.float16)
        in_maps.append(m)

    import os
    trace = os.environ.get("BASS_KERNEL_TRACE", "0") == "1"
    res = run_bass_kernel_spmd(nc, in_maps, list(range(NCORES)), trace=trace)
    kernel._last_exec_ns = res.exec_time_ns
    kernel._last_results = res
    return np.asarray(res.results[0]["out"], np.float32)


kernel._last_exec_ns = None
